# revision 55
# baseline (speedup 1.0000x reference)
"""Block-global self-attention Trainium2 kernel (SPMD over 8 NeuronCores).

Sharding: core c -> batch n = c//4, heads h0 = (c%4)*4 .. h0+3.
Each core receives x = hidden[n] [4096,2048] and wq/wk/wv = W[:, cols]
[2048,512], returns out [4096,512] (its head-column stripe of batch n).

Per-core pipeline:
  P: bf16 projections (host-side bf16 xt/weights, direct DMA) -> qT/kT
     [d,t] + V2 (t-major, 64-row-shifted so local windows are two aligned
     full-K tiles); fp32 approx q-norms -> grid. Local blocks interleave
     with a 1-chunk lag; the last 8 blocks are deferred to cover phase-B
     candidate latency. V2's fused ones-column is zeroed on pad rows so
     pad tokens add neither value nor denominator mass (no score masking).
  A: local block attention; softmax without max-subtraction (|score|<8);
     probs kept unnormalized bf16, 1/denom fused into the final ACT copy.
  B: exact top-62 global tokens via: packed-value (quantized norm + token id
     in low mantissa) 3-level max8 top-96 candidates -> indirect-gather
     X rows -> exact fp32 norms (re-projection with fp32 wq) -> 62nd
     threshold (bos/eos forced slots) -> final index list. Global attention
     is computed transposed (SgT blocks [t,slot] -> exp -> PgT as matmul
     weights -> ctx accumulation over V2 blocks; no transposes), for ALL
     98 candidate slots, overlapped per-head with the exact-norm preps;
     only the final indirect row scatter waits for the selection.
"""
import os
import numpy as np

import concourse.bass as bass
import concourse.bacc as bacc
import concourse.mybir as mybir
from concourse.tile import TileContext, add_dep_helper
from concourse.bass_utils import run_bass_kernel_spmd

F32 = mybir.dt.float32
BF16 = mybir.dt.bfloat16
I32 = mybir.dt.int32

T = 4096
H = 2048
D = 128
NH = 4
KO = H // 128
NB = T // 128
CW = 512
NCHUNK = T // CW
NEG = -30.0
NEGRAW = -30.0 * float(np.sqrt(128.0))  # pre-divided by ACT scale
SCALE = float(1.0 / np.sqrt(128.0))
NCAND = 72
NSLOT = NCAND + 2
GEXP = 512 // NSLOT  # global score blocks per psum bank / exp call
NIDX = 66
DEBUG = bool(int(os.environ.get("KERNEL_DEBUG", "0")))
# PE rest: chained delay-DMAs per chunk boundary; breaks the sustained
# PE-activity streak that trips the P0 2.0 GHz power-state downclock
REST = int(os.environ.get("KERNEL_REST", "0"))


def ts(i, sz):
    return slice(i * sz, (i + 1) * sz)


def _raw(inst):
    return inst.ins if hasattr(inst, "ins") else inst


def build_program():
    nc = bacc.Bacc("TRN2", target_bir_lowering=False, debug=False,
                   enable_asserts=True)
    x_d = nc.dram_tensor("x", (T, H), F32, kind="ExternalInput").ap()
    # chunk-major xt layout: [c, ko, p, t] so each chunk DMA reads a
    # contiguous 512KB slab (sequential DRAM ≈ 5-8x faster than strided)
    xt_d = nc.dram_tensor("xt", (NCHUNK, KO, 128, CW), BF16,
                          kind="ExternalInput").ap()
    wq_d = nc.dram_tensor("wq", (H, NH * D), BF16, kind="ExternalInput").ap()
    wk_d = nc.dram_tensor("wk", (H, NH * D), BF16, kind="ExternalInput").ap()
    wv_d = nc.dram_tensor("wv", (H, NH * D), BF16, kind="ExternalInput").ap()
    wq32_d = nc.dram_tensor("wq32", (H, NH * D), F32, kind="ExternalInput").ap()
    id_d = nc.dram_tensor("ident", (128, 128), F32, kind="ExternalInput").ap()
    out_d = nc.dram_tensor("out", (T, NH * D), F32, kind="ExternalOutput").ap()
    dbg = {}
    if DEBUG:
        dbg["na"] = nc.dram_tensor("dbg_na", (128, NH, 32), F32, kind="ExternalOutput").ap()
        dbg["cand"] = nc.dram_tensor("dbg_cand", (NH, NSLOT), F32, kind="ExternalOutput").ap()
        dbg["ne"] = nc.dram_tensor("dbg_ne", (NH, NSLOT), F32, kind="ExternalOutput").ap()
        dbg["sidx"] = nc.dram_tensor("dbg_sidx", (NSLOT, NH), I32, kind="ExternalOutput").ap()

    with TileContext(nc) as tc:
        const = tc.alloc_tile_pool(name="const", bufs=1)
        res = tc.alloc_tile_pool(name="res", bufs=1)
        dram = tc.alloc_tile_pool(name="dram", bufs=1, space="DRAM")

        ident = const.tile([128, 128], F32)
        nc.sync.dma_start(ident[:], id_d)
        ones_b = const.tile([128, 1], BF16)
        nc.vector.memset(ones_b[:], 1.0)
        ones = const.tile([128, 1], F32)
        nc.vector.memset(ones[:], 1.0)
        iota_g = const.tile([128, NH, 32], F32)
        nc.gpsimd.iota(iota_g[:], pattern=[[0, NH], [1, 32]], base=0,
                       channel_multiplier=32, allow_small_or_imprecise_dtypes=True)

        kT = [res.tile([128, 64 + T + 64], BF16, tag=f"kT{h}", name=f"kT{h}") for h in range(NH)]
        V2 = res.tile([128, NB + 1, NH, D + 1], BF16, tag="V2")
        nagrid = res.tile([128, NH, 32], F32, tag="nagrid")
        wqb = res.tile([128, KO, NH * D], BF16, tag="wqb")
        na_dram = dram.tile([NH, T], F32)

        # ---------------- pools ----------------
        psum = tc.alloc_tile_pool(name="psum", bufs=1, space="PSUM")
        ab = tc.alloc_tile_pool(name="ab", bufs=4)

        def psA2k(nm):   # 2KB f32 one-shot psums
            t = psum.tile([128, 512], F32, tag="A2k", bufs=2, name=nm)
            return t
        def psBLK(nm):   # per-block S + ctx combined
            t = psum.tile([128, 512], F32, tag="blk", bufs=2, name=nm)
            return t
        def psSG(nm):    # global score groups
            t = psum.tile([128, 512], F32, tag="psg", bufs=2, name=nm)
            return t
        def psACC(nm):   # held accumulators
            t = psum.tile([128, 512], F32, tag="ACC", bufs=2, name=nm)
            return t

        # ---------------- interleaved: local attention + global per head ----------------
        out_write_insts = []
        cur_co = [None]

        def local_block(h, b):
            blk = psBLK("blk")
            # S^T halves: [tk(128), tq(128)]; half g covers window pos g*128..,
            # i.e. k tokens [b*128 - 64 + g*128, ...). kT is 64-padded.
            for g in range(2):
                seg = b + g
                nc.tensor.matmul(blk[:, g * 128:(g + 1) * 128],
                                 kT[h][:, seg * 128:seg * 128 + 128],
                                 qT[h][:, ts(b, 128)], start=True, stop=True)
            PT = ab.tile([128, 256], BF16, tag="PT", name="PT", bufs=2)
            nc.scalar.activation(PT[:], blk[:, 0:256], mybir.ActivationFunctionType.Exp,
                                 scale=SCALE)
            pC = blk[:, 256:385]
            nc.tensor.matmul(pC, PT[:, 0:128], V2[:, b, h, :],
                             start=True, stop=False)
            nc.tensor.matmul(pC, PT[:, 128:256], V2[:, b + 1, h, :],
                             start=False, stop=True)
            rc = ab.tile([128, 1], F32, tag="rc", name="rc", bufs=8)
            nc.vector.reciprocal(rc[:], pC[:, 128:129])
            # all 4 heads of a block share one staging tile -> one 256KB
            # out write with 2KB rows (descriptor-rate-bound: 4x fewer DMAs)
            if h == 0:
                cur_co[0] = ab.tile([128, NH, D], F32, tag="co4", name="co4",
                                    bufs=3)
            nc.vector.tensor_scalar_mul(cur_co[0][:, h, :], pC[:, 0:D], rc[:])
            if h == NH - 1:
                w = nc.sync.dma_start(
                    out_d[ts(b, 128), :],
                    cur_co[0][:].rearrange("p h d -> p (h d)"))
                out_write_insts.append(_raw(w))

        def global_scores(h):
            # SgT blocks: psum [t(128), slot]; block jj covers tokens
            # jj*128-64 .. jj*128+63 (kT cols jj*128..+128, V2 block jj).
            # Pad tokens give exp(0)=1 but V2 values AND ones-col are 0
            # there, so they contribute nothing.
            PgT = gbig.tile([128, NB + 1, NSLOT], BF16, tag="PgT",
                            name=f"PgT{h}", bufs=2)
            jj = 0
            while jj <= NB:
                nb = min(GEXP, NB + 1 - jj)
                psg = psSG("psg")
                for gi in range(nb):
                    nc.tensor.matmul(psg[:, gi * NSLOT:(gi + 1) * NSLOT],
                                     kT[h][:, (jj + gi) * 128:(jj + gi + 1) * 128],
                                     qgTh[h][:], start=True, stop=True)
                nc.scalar.activation(
                    PgT[:, jj:jj + nb, :],
                    psg[:, 0:nb * NSLOT].rearrange("p (b s) -> p b s", b=nb),
                    mybir.ActivationFunctionType.Exp, scale=SCALE)
                jj += nb
            return PgT

        def global_ctx(h, PgT):
            pgc = psACC("pgc")[:NSLOT, :D + 1]
            for jj in range(NB + 1):
                nc.tensor.matmul(pgc, PgT[:, jj, :], V2[:, jj, h, :],
                                 start=(jj == 0), stop=(jj == NB),
                                 skip_group_check=True)
            rcg = gw.tile([NSLOT, 1], F32, tag="rcg", bufs=4)
            nc.vector.reciprocal(rcg[:], pgc[:, D:D + 1])
            gco = gw.tile([NSLOT, 128], F32, tag="gco", bufs=4)
            nc.vector.tensor_scalar_mul(gco[:], pgc[:, 0:D], rcg[:])
            return gco

        def scatter_head(h, gco):
            # out viewed as [T*NH, D] rows; sidx encodes token*NH + h so the
            # out AP keeps offset 0 (DynamicAP requirement)
            scat = nc.gpsimd.indirect_dma_start(
                out=out_d.rearrange("t (h d) -> (t h) d", h=NH),
                out_offset=bass.IndirectOffsetOnAxis(ap=sidx_i[:, h:h + 1], axis=0),
                in_=gco[:], in_offset=None,
                bounds_check=T * NH - 1, oob_is_err=False)
            for w in out_write_insts:
                add_dep_helper(_raw(scat), w, reason="scatter after local writes")


        A_DONE = [0]
        # ---------------- phase P ----------------
        na_writes = []
        wkv2 = tc.alloc_tile_pool(name="wkv2", bufs=1)
        wkv = tc.alloc_tile_pool(name="wkv", bufs=1)
        qT = [wkv2.tile([128, T], BF16, tag=f"qT{h}", name=f"qT{h}") for h in range(NH)]
        wkb = wkv.tile([128, KO, NH * D], BF16, tag="wkb")
        wvb = wkv.tile([128, KO, NH * D], BF16, tag="wvb")
        wb = {"q": wqb, "k": wkb, "v": wvb}

        with tc.tile_pool(name="pp", bufs=2) as pp, \
             tc.tile_pool(name="pp1", bufs=1) as pp1:

            xtb_tiles = {}

            def load_xtb(c):
                t = pp1.tile([128, KO, CW], BF16, tag="xtb", bufs=2)
                for kg in range(4):
                    nc.gpsimd.dma_start(
                        t[:, kg * 4:(kg + 1) * 4, :],
                        xt_d[c, kg * 4:(kg + 1) * 4, :, :].rearrange("ko p t -> p ko t"))
                return t

            # ramp order on the Pool queue: wq -> x chunk 0 -> wk -> wv
            # (queues serialize at the DMA arbiter, so issue in need-order)
            wrs = {nm: wd.rearrange("(ko p) m -> p ko m", p=128)
                   for nm, wd in (("q", wq_d), ("k", wk_d), ("v", wv_d))}
            nc.gpsimd.dma_start(wb["q"][:], wrs["q"][:])
            xtb_tiles[0] = load_xtb(0)
            for nm in ("k", "v"):
                nc.gpsimd.dma_start(wb[nm][:], wrs[nm][:])

            for h in range(NH):
                nc.vector.memset(kT[h][:, 0:64], 0.0)
                nc.vector.memset(kT[h][:, 64 + T:], 0.0)
            nc.vector.memset(V2[0:64, 0, :, :], 0.0)
            nc.vector.memset(V2[64:128, NB, :, :], 0.0)
            nc.vector.memset(V2[:, :, :, D:D + 1], 1.0)
            # pad rows contribute neither value nor denominator mass
            nc.vector.memset(V2[0:64, 0, :, D:D + 1], 0.0)
            nc.vector.memset(V2[64:128, NB, :, D:D + 1], 0.0)

            rest_gate = [None]
            for c in range(NCHUNK):
                xtb = xtb_tiles.pop(c) if c in xtb_tiles else load_xtb(c)
                for h in range(NH):
                    for nm, dstT in (("q", qT[h]), ("k", kT[h])):
                        ps = psA2k("psqk")
                        for kb in range(KO):
                            mi = nc.tensor.matmul(ps[:], wb[nm][:, kb, ts(h, D)],
                                                  xtb[:, kb, :], start=(kb == 0),
                                                  stop=(kb == KO - 1))
                            if rest_gate[0] is not None:
                                add_dep_helper(_raw(mi), rest_gate[0],
                                               reason="PE rest gate")
                                rest_gate[0] = None
                        off = 64 if nm == "k" else 0
                        nc.vector.tensor_copy(dstT[:, off + c * CW:off + (c + 1) * CW], ps[:])
                        if nm == "q":
                            sq = pp.tile([128, CW], BF16, tag="sq", bufs=1)
                            nc.vector.tensor_tensor(sq[:], dstT[:, ts(c, CW)],
                                                    dstT[:, ts(c, CW)],
                                                    op=mybir.AluOpType.mult)
                            pn = psA2k("pn")[:1, :]
                            nc.tensor.matmul(pn, ones_b[:], sq[:],
                                             start=True, stop=True)
                            narow = pp.tile([1, CW], F32, tag="narow", bufs=1)
                            nc.vector.tensor_copy(narow[:], pn)
                            w = nc.sync.dma_start(na_dram[h:h + 1, ts(c, CW)], narow[:])
                            na_writes.append(_raw(w))
                for s in range(CW // 128):
                    sg = c * (CW // 128) + s
                    pv = psA2k("psv")
                    for kb in range(KO):
                        nc.tensor.matmul(pv[:], xtb[:, kb, ts(s, 128)],
                                         wb["v"][:, kb, :], start=(kb == 0),
                                         stop=(kb == KO - 1))
                    vt = pp.tile([128, NH * D], BF16, tag="vtmp", bufs=1)
                    nc.vector.tensor_copy(vt[:], pv[:])
                    nc.sync.dma_start(V2[64:128, sg, :, 0:D],
                                      vt[0:64, :].rearrange("p (h d) -> p h d", h=NH))
                    nc.sync.dma_start(V2[0:64, sg + 1, :, 0:D],
                                      vt[64:128, :].rearrange("p (h d) -> p h d", h=NH))
                # interleave ready local-attention blocks (1-chunk lag);
                # hold back the last blocks to cover phase-B latency
                hi = min(4 * c - 2 + 1, NB - 16)
                for b in range(A_DONE[0], hi):
                    for h in range(NH):
                        local_block(h, b)
                A_DONE[0] = max(A_DONE[0], hi)
                if REST and c < NCHUNK - 1:
                    last = None
                    for rr in range(REST):
                        rd = dram.tile([128, 4, CW], BF16, tag="restd")
                        w = nc.gpsimd.dma_start(
                            rd[:], xt_d[c, 0:4, :, :].rearrange("ko p t -> p ko t"))
                        if last is not None:
                            add_dep_helper(_raw(w), last, reason="rest chain")
                        last = _raw(w)
                    rest_gate[0] = last

        # (deferred local blocks are issued after the candidate chain, right
        # behind the gathers — the chain is DVE/DMA-serial, so PE covers the
        # ~13us software-gather latency instead)
        wkv.release()

        # ---------------- phase B part 1: candidates + exact topk ----------------
        gp = tc.alloc_tile_pool(name="gp", bufs=1)
        gbig = tc.alloc_tile_pool(name="gbig", bufs=2)
        gw = tc.alloc_tile_pool(name="gw", bufs=2)
        # w_lo = bf16(wq_fp32 - wqb): together with wqb reconstructs the fp32
        # wq for the exact re-projection using pure bf16 matmuls (hi/lo trick)
        wlo = gbig.tile([128, KO, NH * D], BF16, tag="wlo", bufs=1)
        wq32r = wq32_d.rearrange("(ko p) m -> p ko m", p=128)
        for kb in range(0, KO, 4):
            wstg = gbig.tile([128, 4, NH * D], F32, tag="wstg", bufs=2)
            nc.gpsimd.dma_start(wstg[:], wq32r[:, kb:kb + 4, :])
            nc.vector.tensor_tensor(wlo[:, kb:kb + 4, :], wstg[:],
                                    wqb[:, kb:kb + 4, :],
                                    op=mybir.AluOpType.subtract)
        r = nc.gpsimd.dma_start(nagrid[:],
                              na_dram[:].rearrange("h (p j) -> p h j", p=128))
        for w in na_writes:
            add_dep_helper(_raw(r), w, reason="na grid read after writes")

        m0 = gp.tile([128, NH, 32], F32)
        nc.vector.tensor_scalar(m0[:], iota_g[:], 0.0, scalar2=None,
                                op0=mybir.AluOpType.is_equal)
        m1 = gp.tile([128, NH, 32], F32)
        nc.vector.tensor_scalar(m1[:], iota_g[:], 4095.0, scalar2=None,
                                op0=mybir.AluOpType.is_equal)
        nc.vector.tensor_tensor(m0[:], m0[:], m1[:], op=mybir.AluOpType.add)
        nagp = gp.tile([128, NH, 32], F32)
        nc.vector.tensor_tensor(nagp[:], nagrid[:], m0[:], op=mybir.AluOpType.mult)
        nc.vector.tensor_tensor(nagp[:], nagrid[:], nagp[:], op=mybir.AluOpType.subtract)
        nc.vector.tensor_scalar_mul(m0[:], m0[:], 1.0e6)
        nc.vector.tensor_tensor(nagp[:], nagp[:], m0[:], op=mybir.AluOpType.subtract)
        pk = gp.tile([128, NH, 32], F32)
        nc.vector.tensor_scalar_mul(pk[:], nagp[:], 4.0)
        pki = gp.tile([128, NH, 32], I32)
        nc.vector.tensor_copy(pki[:], pk[:])
        nc.vector.tensor_copy(pk[:], pki[:])
        nc.vector.tensor_scalar_mul(pk[:], pk[:], 0.125)
        io16 = gp.tile([128, NH, 32], F32)
        nc.vector.tensor_scalar_mul(io16[:], iota_g[:], 2.0 ** -16)
        nc.vector.tensor_tensor(pk[:], pk[:], io16[:], op=mybir.AluOpType.add)
        pk2 = pk[:].rearrange("p h j -> p (h j)")

        # L1: per-partition (32 tokens) top-8 per head — single max8, no
        # replace rounds (top-62 tokens never exceed 4 per partition here)
        cand1 = gp.tile([128, NH * 8], F32)
        for h in range(NH):
            nc.vector.max(out=cand1[:, h * 8:(h + 1) * 8], in_=pk2[:, ts(h, 32)])
        # regroup via one DRAM bounce: row (h,i) collects partitions i mod 16
        c1d = dram.tile([8, 16, NH, 8], F32)  # [g, i, h, j]
        w1 = nc.gpsimd.dma_start(c1d[:].rearrange("g i h j -> (g i) (h j)"),
                               cand1[:])
        lvl2 = gp.tile([64, 64], F32)
        for h in range(NH):
            r2 = nc.gpsimd.dma_start(
                lvl2[h * 16:(h + 1) * 16, :].rearrange("i (g j) -> i g j", g=8),
                c1d[:, :, h, :].rearrange("g i j -> i g j"))
            add_dep_helper(_raw(r2), _raw(w1), reason="lvl2 read after write")
        cand2 = gp.tile([64, 24], F32)
        for rr in range(3):
            nc.vector.max(out=cand2[:, ts(rr, 8)], in_=lvl2[:])
            if rr < 2:
                nc.vector.match_replace(out=lvl2[:], in_to_replace=cand2[:, ts(rr, 8)],
                                        in_values=lvl2[:], imm_value=-1e30)
        c2d = dram.tile([64, 24], F32)
        w2 = nc.gpsimd.dma_start(c2d[:], cand2[:])
        lvl3 = gp.tile([NH, 384], F32)
        r3 = nc.gpsimd.dma_start(lvl3[:],
                               c2d[:].rearrange("(h p) c -> h (p c)", h=NH))
        add_dep_helper(_raw(r3), _raw(w2), reason="lvl3 read after write")
        tops = gp.tile([NH, NCAND], F32)
        for rr in range(NCAND // 8):
            nc.vector.max(out=tops[:, ts(rr, 8)], in_=lvl3[:])
            if rr < NCAND // 8 - 1:
                nc.vector.match_replace(out=lvl3[:], in_to_replace=tops[:, ts(rr, 8)],
                                        in_values=lvl3[:], imm_value=-1e30)

        def decode_t(dst, src, n):
            t1 = gp.tile([NH, n], F32, tag="dec1")
            nc.vector.tensor_scalar_mul(t1[:], src, 8.0)
            t1i = gp.tile([NH, n], I32, tag="dec2")
            nc.vector.tensor_copy(t1i[:], t1[:])
            t1f = gp.tile([NH, n], F32, tag="dec3")
            nc.vector.tensor_copy(t1f[:], t1i[:])
            nc.vector.tensor_tensor(t1[:], t1[:], t1f[:], op=mybir.AluOpType.subtract)
            nc.vector.tensor_scalar_mul(dst, t1[:], 8192.0)

        cand_t = gp.tile([NH, NSLOT], F32)
        decode_t(cand_t[:, 0:NCAND], tops[:], NCAND)
        nc.vector.memset(cand_t[:, NCAND:NCAND + 1], 0.0)
        nc.vector.memset(cand_t[:, NCAND + 1:NSLOT], 4095.0)
        if DEBUG:
            nc.sync.dma_start(dbg["cand"], cand_t[:])

        pslt = psA2k("pslt")[:NSLOT, :NH]
        nc.tensor.transpose(pslt, cand_t[:], ident[:NH, :NH])
        ctf = gp.tile([NSLOT, NH], F32)
        nc.vector.tensor_copy(ctf[:], pslt)
        cti = gp.tile([NSLOT, NH], I32)
        nc.vector.tensor_copy(cti[:], ctf[:])

        # candidate-row gathers for all heads upfront (Pool queue). These are
        # software DMAs (~13us each); the deferred local blocks issued right
        # after keep PE busy while the first gather lands.
        xsels = []
        for h in range(NH):
            xsel = gbig.tile([128, H], F32, tag="xsel", bufs=2, name=f"xsel{h}")
            nc.gpsimd.indirect_dma_start(
                out=xsel[0:NSLOT, :], out_offset=None, in_=x_d,
                in_offset=bass.IndirectOffsetOnAxis(ap=cti[:, h:h + 1], axis=0))
            xsels.append(xsel)

        for b in range(A_DONE[0], NB):
            for h in range(NH):
                local_block(h, b)

        ne_all = gp.tile([NH, NSLOT], F32)
        qgTh = [None] * NH

        def prep_head(h):
            # exact re-projection of the candidate q rows (selection must
            # match the reference's fp32 norms bit-closely): split x and wq
            # into bf16 hi+lo; q = xh@wh + xl@wh + xh@wl (xl@wl ~ 1e-6 rel,
            # dropped) — all bf16 matmuls at full PE rate
            xh = gbig.tile([128, KO, NSLOT], BF16, tag="xcth", bufs=2)
            xl = gbig.tile([128, KO, NSLOT], BF16, tag="xctl", bufs=2)
            for kb in range(KO):
                ptx = psA2k("ptx")[:, :NSLOT]
                nc.tensor.transpose(ptx, xsels[h][0:NSLOT, ts(kb, 128)],
                                    ident[:NSLOT, :NSLOT])
                nc.vector.tensor_copy(xh[:, kb, :], ptx)
                nc.vector.tensor_tensor(xl[:, kb, :], ptx, xh[:, kb, :],
                                        op=mybir.AluOpType.subtract)
            pqc = psACC("pqc")[:, :NSLOT]
            for i, (w_, x_) in enumerate(((wqb, xh), (wqb, xl), (wlo, xh))):
                for kb in range(KO):
                    nc.tensor.matmul(pqc, w_[:, kb, ts(h, D)], x_[:, kb, :],
                                     start=(i == 0 and kb == 0),
                                     stop=(i == 2 and kb == KO - 1),
                                     skip_group_check=True)
            qcf = gw.tile([128, NSLOT], F32, tag="qcf")
            nc.vector.tensor_copy(qcf[:], pqc)
            qgTh[h] = gbig.tile([128, NSLOT], BF16, tag=f"qgT{h}", name=f"qgT{h}")
            nc.vector.tensor_copy(qgTh[h][:], qcf[:])
            sqc = gw.tile([128, NSLOT], F32, tag="sqc")
            nc.vector.tensor_tensor(sqc[:], qcf[:], qcf[:], op=mybir.AluOpType.mult)
            pne = psA2k("pne")[:1, :NSLOT]
            nc.tensor.matmul(pne, ones[:], sqc[:], start=True, stop=True)
            nerow = gw.tile([1, NSLOT], F32, tag="nerow")
            nc.vector.tensor_copy(nerow[:], pne)
            nc.gpsimd.dma_start(ne_all[h:h + 1, :], nerow[:])

        # software-pipelined: head h+1 preps while head h runs on PE
        PgTs = [None] * NH
        gcos = [None] * NH
        prep_head(0)
        PgTs[0] = global_scores(0)
        prep_head(1)
        gcos[0] = global_ctx(0, PgTs[0])
        PgTs[1] = global_scores(1)
        prep_head(2)
        gcos[1] = global_ctx(1, PgTs[1])
        PgTs[2] = global_scores(2)
        prep_head(3)
        if DEBUG:
            nc.sync.dma_start(dbg["ne"], ne_all[:])

        # threshold/selection chain (DVE; overlaps the PE work above)
        ne_work = gp.tile([NH, NSLOT], F32)
        nc.vector.tensor_copy(ne_work[:], ne_all[:])
        tops_e = gp.tile([NH, 64], F32)
        for rr in range(8):
            nc.vector.max(out=tops_e[:, ts(rr, 8)], in_=ne_work[:])
            if rr < 7:
                nc.vector.match_replace(out=ne_work[:], in_to_replace=tops_e[:, ts(rr, 8)],
                                        in_values=ne_work[:], imm_value=-1e30)
        theta = gp.tile([NH, 1], F32)
        nc.vector.tensor_copy(theta[:], tops_e[:, 61:62])

        # sel over the 98 slots; specials (slots 96/97) always selected
        sel = gp.tile([NH, NSLOT], F32)
        nc.vector.tensor_tensor(sel[:], ne_all[:], theta[:].to_broadcast([NH, NSLOT]),
                                op=mybir.AluOpType.is_ge)
        nc.vector.memset(sel[:, NCAND:NSLOT], 1.0)
        # scatter idx per slot: cand_t if selected else OOB (100000)
        sidx_f = gp.tile([NH, NSLOT], F32)
        nc.vector.tensor_scalar(sidx_f[:], sel[:], -1.0, scalar2=None,
                                op0=mybir.AluOpType.add)
        nc.vector.tensor_scalar_mul(sidx_f[:], sidx_f[:], -100000.0)
        nc.vector.tensor_tensor(sidx_f[:], sidx_f[:], cand_t[:], op=mybir.AluOpType.add)
        # fold head into the row index: row = token*NH + h (see scatter_head)
        nc.vector.tensor_scalar_mul(sidx_f[:], sidx_f[:], float(NH))
        hcol = gp.tile([NH, 1], F32)
        nc.gpsimd.iota(hcol[:], pattern=[[0, 1]], base=0, channel_multiplier=1,
                       allow_small_or_imprecise_dtypes=True)
        nc.vector.tensor_tensor(sidx_f[:], sidx_f[:],
                                hcol[:].to_broadcast([NH, NSLOT]),
                                op=mybir.AluOpType.add)
        p_ = psA2k("ptr")[:NSLOT, :NH]
        nc.tensor.transpose(p_, sidx_f[:], ident[:NH, :NH])
        sf1 = gp.tile([NSLOT, NH], F32)
        nc.vector.tensor_copy(sf1[:], p_)
        sidx_i = gp.tile([NSLOT, NH], I32)
        nc.vector.tensor_copy(sidx_i[:], sf1[:])
        if DEBUG:
            nc.sync.dma_start(dbg["sidx"], sidx_i[:])
            nc.sync.dma_start(dbg["na"], nagrid[:])

        gcos[2] = global_ctx(2, PgTs[2])
        PgTs[3] = global_scores(3)
        gcos[3] = global_ctx(3, PgTs[3])
        for h in range(NH):
            scatter_head(h, gcos[h])

        gw.release()
        gbig.release()
        gp.release()
        wkv2.release()
        ab.release()
        psum.release()
        dram.release()
        res.release()
        const.release()

    nc.finalize()
    return nc


_NC_CACHE = None


def make_in_maps(inputs):
    import ml_dtypes
    BF = ml_dtypes.bfloat16
    hs = np.ascontiguousarray(np.asarray(inputs["hidden_states"], dtype=np.float32))
    Wq = np.ascontiguousarray(np.asarray(inputs["Wq"], dtype=np.float32))
    Wk = np.ascontiguousarray(np.asarray(inputs["Wk"], dtype=np.float32))
    Wv = np.ascontiguousarray(np.asarray(inputs["Wv"], dtype=np.float32))
    ident = np.eye(128, dtype=np.float32)
    # chunk-major [c, ko, p, t] layout (contiguous per-chunk slabs)
    xts_host = [
        np.ascontiguousarray(
            hs[n].T.astype(BF).reshape(16, 128, 8, 512).transpose(2, 0, 1, 3))
        for n in range(2)
    ]
    in_maps = []
    for c in range(8):
        n = c // 4
        h0 = (c % 4) * NH
        cols = slice(h0 * D, (h0 + NH) * D)
        in_maps.append({
            "x": hs[n],
            "xt": xts_host[n],
            "wq": np.ascontiguousarray(Wq[:, cols].astype(BF)),
            "wk": np.ascontiguousarray(Wk[:, cols].astype(BF)),
            "wv": np.ascontiguousarray(Wv[:, cols].astype(BF)),
            "wq32": np.ascontiguousarray(Wq[:, cols]),
            "ident": ident,
        })
    return in_maps


def kernel(**inputs):
    global _NC_CACHE
    if _NC_CACHE is None:
        _NC_CACHE = build_program()
    nc = _NC_CACHE
    in_maps = make_in_maps(inputs)
    res = run_bass_kernel_spmd(nc, in_maps, core_ids=list(range(8)))
    out = np.zeros((2, T, H), np.float32)
    for c in range(8):
        n = c // 4
        h0 = (c % 4) * NH
        out[n, :, h0 * D:(h0 + NH) * D] = res.results[c]["out"]
    return out

